# revision 8
# baseline (speedup 1.0000x reference)
"""Bidirectional LSTM layer on 8 TRN2 NeuronCores (Bass/Tile).

Problem: B=64, S=512, I=H=1024, fp32.
  hs_fw = LSTM_fw(x), hs_bw = reverse(LSTM_bw(reverse(x))), out = concat -> [B,S,2H]

Sharding: pure SPMD, one program. Cores 0-3: forward dir, batch chunks of 16.
Cores 4-7: backward dir (host feeds time-reversed x, host un-reverses output),
batch chunks of 16. bw_h_mask is folded into W_hh_bw on the host.

Single fused loop (v2): the input projection GEMM is interleaved into the
recurrent scan one [128,512] output chunk per step (8 kc matmuls), so the PE
fills the elementwise-tail gap and stays at full pump (HAM K=8/8).

Per step t:
  - 64 h-MMs: gates += hT(t-1) @ W_hh', 4-way PE column tiling (groups j own
    psum partitions 32j..32j+32), bank-major (bank0 = [i|g~], bank1 = [o|f]),
    even kc chunks first (they only need hT half 0).
  - 2 f32r inject MMs put xp(t+1) into the next psum pair (start=True).
  - 8 proj MMs: one n8-chunk of row-tile t//8 + LEAD; bias-add on DVE; staged
    to DRAM f32r.
  - elementwise tail split into cell halves so transpose(h half0) can feed
    step t+1's first (even-kc) h-MMs while half1 finishes:
    ACT: sig(i), tanh(g~) [during bank1 MMs], sig(f0), sig(f1), sig(o),
         tanh(c0), tanh(c1), copy hT1
    DVE: t1=i*g~ [during bank1], c0, c1, h0, h1 (bf16), cast hT0
    Pool: t2_0=f0*c_prev0, t2_1
  - 2 bf16 PE transposes of h halves -> hT stationaries for t+1.
  - hs[t] written bf16 (host upcasts).

Weight row permutation: PyTorch gate order (i,f,g,o) x 1024 cells ->
(j, g', c) with g' in (i, g~, o, f), j=cell//256, c=cell%256.
"""

import os
import sys

sys.path.insert(0, "/opt/trn_rl_repo")

from contextlib import ExitStack

import numpy as np

import concourse.bass as bass
import concourse.tile as tile
from concourse import bacc, mybir
from concourse.tile_rust import add_dep_helper

F32 = mybir.dt.float32
F32R = mybir.dt.float32r
BF16 = mybir.dt.bfloat16
AF = mybir.ActivationFunctionType

B_LOC = 16  # batch per core
H = 1024
I = 1024
KC = I // 128  # 8 contraction chunks
NG = 4  # column-tile groups
CPG = H // NG  # cells per group = 256
GATE_PERM = [0, 2, 3, 1]  # new gate order (i, g~, o, f) from pytorch (i, f, g, o)
KC_ORDER = [0, 2, 4, 6, 1, 3, 5, 7]  # even kc first (need only hT half 0)
LEAD = 2  # proj row-tiles computed ahead of consumption


# ----------------------------------------------------------------- host prep
def perm_rows(w4h: np.ndarray) -> np.ndarray:
    """Permute [4H, K] gate-major rows (pytorch i,f,g,o) -> (j, g', c) order."""
    k = w4h.shape[1]
    w = w4h.reshape(4, NG, CPG, k)[GATE_PERM]  # [g', j, c, K]
    w = w.transpose(1, 0, 2, 3)  # [j, g', c, K]
    return np.ascontiguousarray(w.reshape(4 * H, k))


def _bf16(a):
    import ml_dtypes
    return a.astype(ml_dtypes.bfloat16)


def prep_w_ih(w_ih: np.ndarray) -> np.ndarray:
    """[4H, I] -> [8, 128, 4H]  ([kc, p, n]) for SBUF rhs streaming."""
    wp = perm_rows(w_ih)  # [4096n, 1024i]
    return _bf16(np.ascontiguousarray(wp.T.reshape(I // 128, 128, 4 * H)))


def prep_w_hh(w_hh: np.ndarray) -> np.ndarray:
    """[4H, H] -> [8, 128, 4, 1024] ([kc, p, j, g'*c])."""
    wp = perm_rows(w_hh)  # [4096n=(j,g',c), 1024k]
    wt = wp.T.reshape(H // 128, 128, NG, 4 * CPG)
    return _bf16(np.ascontiguousarray(wt))


def prep_bias(b: np.ndarray) -> np.ndarray:
    return np.ascontiguousarray(perm_rows(b.reshape(4 * H, 1)).reshape(4 * H))


def prep_x(x_shard: np.ndarray, reverse_time: bool) -> np.ndarray:
    """[16, S, 1024] -> xT [1024, S*16] (i, t*b) t-major."""
    if reverse_time:
        x_shard = x_shard[:, ::-1, :]
    s = x_shard.shape[1]
    xt = x_shard.transpose(2, 1, 0)  # [i, t, b]
    return _bf16(np.ascontiguousarray(xt.reshape(I, s * B_LOC)))


# ------------------------------------------------------------------- builder
def build_program(S: int) -> bacc.Bacc:
    nc = bacc.Bacc(
        "TRN2",
        target_bir_lowering=False,
        debug=False,
        enable_asserts=True,
    )

    TB = S * B_LOC  # rows of the proj GEMM
    assert TB % 128 == 0
    NMT = TB // 128  # proj row tiles (= S // 8)
    assert NMT * 8 == S

    XPDT = BF16 if os.environ.get("LSTM_XP", "bf16") == "bf16" else F32R

    xT = nc.dram_tensor("xT", [I, TB], BF16, kind="ExternalInput").ap()
    w_ih = nc.dram_tensor("w_ih", [KC, 128, 4 * H], BF16, kind="ExternalInput").ap()
    w_hh = nc.dram_tensor("w_hh", [KC, 128, NG, 4 * CPG], BF16, kind="ExternalInput").ap()
    bias = nc.dram_tensor("bias", [4 * H], F32, kind="ExternalInput").ap()
    ident = nc.dram_tensor("ident", [32, 32], F32, kind="ExternalInput").ap()
    hs = nc.dram_tensor("hs", [S, B_LOC, H], BF16, kind="ExternalOutput").ap()
    # xp staged in DRAM: [t, j, b, g', c] single plane
    xp_d = nc.dram_tensor("xp_stage", [S, NG, B_LOC, 4, CPG], XPDT, kind="Internal").ap()
    XPST = NG * B_LOC * 4 * CPG  # xp elements per t

    with tile.TileContext(nc) as tc, ExitStack() as ctx:
        # PE executes serially; pin the scheduler to our emission order for all
        # PE instructions so psum accumulation-group start/stop semantics can't
        # be violated by hoisting (scheduling-only deps, no semaphores).
        pe_prev = [None]

        def pe(bi):
            if pe_prev[0] is not None:
                add_dep_helper(bi.ins, pe_prev[0].ins, sync=False,
                               reason="PE emission order")
            pe_prev[0] = bi
            return bi

        # =================== constants that live for the whole kernel ======
        const_pool = ctx.enter_context(tc.tile_pool(name="consts", bufs=1))

        # full 128x128 identity (bf16) for PE transpose (block-diagonal eye32)
        ident_f32 = const_pool.tile([128, 128], F32)
        nc.vector.memset(ident_f32[:], 0.0)
        for j in range(NG):
            nc.sync.dma_start(
                out=ident_f32[32 * j : 32 * (j + 1), 32 * j : 32 * (j + 1)],
                in_=ident,
            )
        ident_full = const_pool.tile([128, 128], BF16)
        nc.vector.tensor_copy(ident_full[:], ident_f32[:])
        # combined selection matrix: esel[16j+b, 32j+b] = 1 (b < 16), f32r.
        # One full-width MM injects xp into all 4 psum groups and opens the
        # accumulation group for the whole bank in a single start=True.
        esel_f32 = const_pool.tile([64, 128], F32)
        nc.vector.memset(esel_f32[:], 0.0)
        for j in range(NG):
            nc.sync.dma_start(
                out=esel_f32[16 * j : 16 * (j + 1), 32 * j : 32 * (j + 1)],
                in_=ident[0:16, :],
            )
        esel_r = const_pool.tile([64, 128], XPDT)
        nc.vector.tensor_copy(esel_r[:], esel_f32[:])

        # ======================= persistent weights ========================
        wih_sb = const_pool.tile([128, KC, 4 * H], BF16)
        for kc in range(KC):
            nc.sync.dma_start(out=wih_sb[:, kc, :], in_=w_ih[kc])
        whh_sb = const_pool.tile([128, KC, NG, 4 * CPG], BF16)
        for kc in range(KC):
            nc.sync.dma_start(out=whh_sb[:, kc, :, :], in_=w_hh[kc])
        bias_sb = const_pool.tile([128, 4 * H], F32)
        nc.sync.dma_start(
            out=bias_sb[:],
            in_=bass.AP(tensor=bias.tensor, offset=0, ap=[[0, 128], [1, 4 * H]]),
        )

        # ============================ pools ================================
        xt_pool = ctx.enter_context(tc.tile_pool(name="xt", bufs=2))
        pj_psum = ctx.enter_context(tc.tile_pool(name="pj_ps", bufs=2, space="PSUM"))
        pj_stage = ctx.enter_context(tc.tile_pool(name="pj_st", bufs=3))
        xp_pool = ctx.enter_context(tc.tile_pool(name="xp", bufs=3))
        g_psum = ctx.enter_context(tc.tile_pool(name="gates_ps", bufs=4, space="PSUM"))
        t_psum = ctx.enter_context(tc.tile_pool(name="tr_ps", bufs=2, space="PSUM"))
        a_pool = ctx.enter_context(tc.tile_pool(name="acts", bufs=2))
        c_pool = ctx.enter_context(tc.tile_pool(name="cell", bufs=2))
        h_pool = ctx.enter_context(tc.tile_pool(name="hid", bufs=2))
        tmp_pool = ctx.enter_context(tc.tile_pool(name="tmp", bufs=2))
        hT_pool = ctx.enter_context(tc.tile_pool(name="hT", bufs=4))

        # ======================= proj chunk helpers ========================
        xt_cur = [None]  # current row-tile's xT chunks [128, KC*128]

        def load_xt(mt):
            xt_t = xt_pool.tile([128, KC * 128], BF16, tag="xt")
            for kc in range(KC):
                nc.sync.dma_start(
                    out=xt_t[:, kc * 128 : (kc + 1) * 128],
                    in_=xT[kc * 128 : (kc + 1) * 128, mt * 128 : (mt + 1) * 128],
                )
            xt_cur[0] = xt_t

        def emit_proj_mms(mt, n8, kcs, ps=None):
            """kcs-slice of the 8 kc MMs of one [128,512] chunk of row-tile mt."""
            if ps is None:
                ps = pj_psum.tile([128, 512], F32, tag="pjps")
            for kc in kcs:
                pe(nc.tensor.matmul(
                    ps[:],
                    xt_cur[0][:, kc * 128 : (kc + 1) * 128],
                    wih_sb[:, kc, n8 * 512 : (n8 + 1) * 512],
                    start=(kc == 0),
                    stop=(kc == KC - 1),
                    skip_group_check=True,
                ))
            return ps

        def emit_proj_store(mt, n8, ps):
            stg = pj_stage.tile([128, 512], XPDT, tag="pjstg")
            nc.vector.tensor_add(
                stg[:], ps[:], bias_sb[:, n8 * 512 : (n8 + 1) * 512]
            )
            t0 = mt * 8  # first t of this row-tile (8 t's x 16 b's)
            j, g0 = n8 // 2, (n8 % 2) * 2
            dst = bass.AP(
                tensor=xp_d.tensor,
                offset=t0 * XPST + j * (B_LOC * 4 * CPG) + g0 * CPG,
                ap=[
                    [XPST, 8],  # t
                    [4 * CPG, B_LOC],  # b
                    [CPG, 2],  # g'
                    [1, CPG],  # c
                ],
            )
            src = bass.AP(
                tensor=stg.tensor,
                offset=stg[:].offset,
                ap=[[512, 128], [CPG, 2], [1, CPG]],
            )
            nc.sync.dma_start(out=dst, in_=src)

        def inject_xp(t):
            """Load xp(t) and open the psum pair for step t with it."""
            xp_t = xp_pool.tile([64, 4 * CPG], XPDT, tag="xp")
            nc.sync.dma_start(
                out=xp_t[:],
                in_=bass.AP(
                    tensor=xp_d.tensor,
                    offset=t * XPST,
                    ap=[[4 * CPG, NG * B_LOC], [1, 4 * CPG]],
                ),
            )
            Gb = [g_psum.tile([128, 512], F32, tag="G", name=f"G{nh}")
                  for nh in range(2)]
            for nh in range(2):
                pe(nc.tensor.matmul(
                    Gb[nh][:],
                    esel_r[:],
                    xp_t[:, nh * 512 : (nh + 1) * 512],
                    start=True,
                    stop=(t == 0),  # t=0 has no h-MMs
                    skip_group_check=True,
                ))
            return Gb

        # ========================= prologue ================================
        # proj row-tiles 0..LEAD-1 so xp(t) exists for the first 8*LEAD steps
        for mt in range(LEAD):
            load_xt(mt)
            for n8 in range(8):
                ps = emit_proj_mms(mt, n8, range(KC))
                emit_proj_store(mt, n8, ps)

        next_Gb = inject_xp(0)

        # ========================= the scan ================================
        c_prev = None
        hT_prev = None  # [hT_half0, hT_half1] stationary tiles [128,128] bf16
        for t in range(S):
            Gb = next_Gb

            # ---- h-MMs: bank-major, even kc chunks first
            if hT_prev is not None:
                for nh in range(2):
                    for kci, kc in enumerate(KC_ORDER):
                        jc, half = kc // 2, kc % 2
                        for j in range(NG):
                            pe(nc.tensor.matmul(
                                Gb[nh][32 * j : 32 * (j + 1), :],
                                hT_prev[half][:, 32 * jc : 32 * (jc + 1)],
                                whh_sb[:, kc, j, nh * 512 : (nh + 1) * 512],
                                start=False,
                                stop=(kci == KC - 1),
                                tile_position=(0, 32 * j),
                                skip_group_check=True,
                            ))

            # ---- bank0 activations (overlap bank1 h-MMs): i sigmoid, g~ tanh
            A = a_pool.tile([128, 4 * CPG], F32, tag="A")
            nc.scalar.activation(A[:, 0:CPG], Gb[0][:, 0:CPG], AF.Sigmoid)
            nc.scalar.activation(A[:, CPG : 2 * CPG], Gb[0][:, CPG : 2 * CPG], AF.Tanh)
            t1 = tmp_pool.tile([128, CPG], F32, tag="T1")
            nc.gpsimd.tensor_mul(t1[:], A[:, 0:CPG], A[:, CPG : 2 * CPG])

            # ---- open next psum pair with xp(t+1); proj part A fills the
            # PE gap until h half 0 is ready for its transpose
            if t + 1 < S:
                next_Gb = inject_xp(t + 1)
            ck = t // 8 + LEAD
            pj_ps = None
            if ck < NMT:
                n8 = t % 8
                if n8 == 0:
                    load_xt(ck)
                pj_ps = emit_proj_mms(ck, n8, range(0, 4))

            # ---- bank1 activations: f halves first (feed c chain), then o
            ao = A[:, 2 * CPG : 3 * CPG]
            af = A[:, 3 * CPG : 4 * CPG]
            nc.scalar.activation(A[:, 3 * CPG : 3 * CPG + 128],
                                 Gb[1][:, CPG : CPG + 128], AF.Sigmoid)
            nc.scalar.activation(A[:, 3 * CPG + 128 : 4 * CPG],
                                 Gb[1][:, CPG + 128 : 2 * CPG], AF.Sigmoid)
            nc.scalar.activation(ao, Gb[1][:, 0:CPG], AF.Sigmoid)

            # ---- cell/hidden update, split into cell halves; DVE chain runs
            # t2/c/h back-to-back, ACT interleaves tanh
            c_new = c_pool.tile([128, CPG], F32, tag="C")
            tc_t = tmp_pool.tile([128, CPG], F32, tag="TC")
            h_new = h_pool.tile([128, CPG], BF16, tag="Hb")
            t2 = tmp_pool.tile([128, CPG], F32, tag="T2")
            pts = []
            for hf in range(2):
                sl = slice(128 * hf, 128 * (hf + 1))
                if c_prev is None:
                    nc.vector.tensor_copy(c_new[:, sl], t1[:, sl])
                else:
                    nc.vector.tensor_mul(t2[:, sl], af[:, sl], c_prev[:, sl])
                    nc.vector.tensor_add(c_new[:, sl], t1[:, sl], t2[:, sl])
                nc.scalar.activation(tc_t[:, sl], c_new[:, sl], AF.Tanh)
                nc.vector.tensor_mul(h_new[:, sl], ao[:, sl], tc_t[:, sl])
                # transpose this half for next step's stationary operand
                if t < S - 1:
                    pt = t_psum.tile([128, 128], BF16, tag="PT")
                    pe(nc.tensor.transpose(pt[:], h_new[:, sl], ident_full[:]))
                    pts.append(pt)
            c_prev = c_new

            # ---- proj part B rides after the transposes, during the hT copies
            if pj_ps is not None:
                emit_proj_mms(ck, t % 8, range(4, 8), pj_ps)

            # ---- hT psum->SBUF copies: cast0 on DVE (emitted after the h
            # chain so it can't block it), copy1 on ACT
            if t < S - 1:
                ht0 = hT_pool.tile([128, 128], BF16, tag="hT", name="hT0")
                nc.vector.tensor_copy(ht0[:], pts[0][:])
                ht1 = hT_pool.tile([128, 128], BF16, tag="hT", name="hT1")
                nc.scalar.copy(ht1[:], pts[1][:])
                hT_prev = [ht0, ht1]

            # ---- proj chunk epilogue (bias add + store), DVE last
            if pj_ps is not None:
                emit_proj_store(ck, t % 8, pj_ps)

            # ---- write hs[t]  (group j partitions -> hs[t, :, 256j:256j+256])
            for j in range(NG):
                nc.sync.dma_start(
                    out=bass.AP(
                        tensor=hs.tensor,
                        offset=t * (B_LOC * H) + j * CPG,
                        ap=[[H, B_LOC], [1, CPG]],
                    ),
                    in_=h_new[32 * j : 32 * j + 16, :],
                )

    nc.compile()
    return nc


# ------------------------------------------------------------------ wrapper
def make_in_maps(x, W_ih_fw, W_hh_fw, b_fw, W_ih_bw, W_hh_bw, b_bw, bw_h_mask):
    """Full inputs -> list of 8 per-core input dicts."""
    x = np.asarray(x, np.float32)
    eye = np.eye(32, dtype=np.float32)
    whh_bw_m = np.asarray(W_hh_bw, np.float32) * np.asarray(
        bw_h_mask, np.float32
    ).reshape(1, H)
    fw = dict(
        w_ih=prep_w_ih(np.asarray(W_ih_fw, np.float32)),
        w_hh=prep_w_hh(np.asarray(W_hh_fw, np.float32)),
        bias=prep_bias(np.asarray(b_fw, np.float32)),
    )
    bw = dict(
        w_ih=prep_w_ih(np.asarray(W_ih_bw, np.float32)),
        w_hh=prep_w_hh(whh_bw_m),
        bias=prep_bias(np.asarray(b_bw, np.float32)),
    )
    in_maps = []
    for core in range(8):
        rev = core >= 4
        wd = bw if rev else fw
        bc = (core % 4) * B_LOC
        in_maps.append(
            dict(
                xT=prep_x(x[bc : bc + B_LOC], rev),
                ident=eye,
                **wd,
            )
        )
    return in_maps


def assemble_output(results, S):
    """8 per-core 'hs' [S,16,H] (bf16) -> [B, S, 2H] f32."""
    out = np.empty((64, S, 2 * H), np.float32)
    for core in range(8):
        hs = np.asarray(results[core]["hs"]).astype(np.float32)
        bc = (core % 4) * B_LOC
        if core < 4:
            out[bc : bc + B_LOC, :, :H] = hs.transpose(1, 0, 2)
        else:
            out[bc : bc + B_LOC, :, H:] = hs[::-1].transpose(1, 0, 2)
    return out


# ======================================================================
# harness entry point
# ======================================================================
_PROG_CACHE = {}


def _get_program(S):
    if S not in _PROG_CACHE:
        _PROG_CACHE[S] = build_program(S)
    return _PROG_CACHE[S]


def kernel(x, W_ih_fw, W_hh_fw, b_fw, W_ih_bw, W_hh_bw, b_bw, bw_h_mask):
    """Full (unsharded) inputs -> full [B, S, 2H] output on 8 TRN2
    NeuronCores (cores 0-3 forward, 4-7 backward, batch-sharded by 16)."""
    from concourse.bass_utils import run_bass_kernel_spmd

    x = np.asarray(x, np.float32)
    S = x.shape[1]
    nc = _get_program(S)
    in_maps = make_in_maps(
        x, W_ih_fw, W_hh_fw, b_fw, W_ih_bw, W_hh_bw, b_bw, bw_h_mask
    )
    res = run_bass_kernel_spmd(nc, in_maps, core_ids=list(range(8)))
    return assemble_output(res.results, S)


# revision 9
# speedup vs baseline: 1.0583x; 1.0583x over previous
"""Bidirectional LSTM layer on 8 TRN2 NeuronCores (Bass/Tile).

Problem: B=64, S=512, I=H=1024, fp32.
  hs_fw = LSTM_fw(x), hs_bw = reverse(LSTM_bw(reverse(x))), out = concat -> [B,S,2H]

Sharding: pure SPMD, one program. Cores 0-3: forward dir, batch chunks of 16.
Cores 4-7: backward dir (host feeds time-reversed x, host un-reverses output),
batch chunks of 16. bw_h_mask is folded into W_hh_bw on the host.

Single fused loop (v2): the input projection GEMM is interleaved into the
recurrent scan one [128,512] output chunk per step (8 kc matmuls), so the PE
fills the elementwise-tail gap and stays at full pump (HAM K=8/8).

Per step t:
  - 64 h-MMs: gates += hT(t-1) @ W_hh', 4-way PE column tiling (groups j own
    psum partitions 32j..32j+32), bank-major (bank0 = [i|g~], bank1 = [o|f]),
    even kc chunks first (they only need hT half 0).
  - 2 f32r inject MMs put xp(t+1) into the next psum pair (start=True).
  - 8 proj MMs: one n8-chunk of row-tile t//8 + LEAD; bias-add on DVE; staged
    to DRAM f32r.
  - elementwise tail split into cell halves so transpose(h half0) can feed
    step t+1's first (even-kc) h-MMs while half1 finishes:
    ACT: sig(i), tanh(g~) [during bank1 MMs], sig(f0), sig(f1), sig(o),
         tanh(c0), tanh(c1), copy hT1
    DVE: t1=i*g~ [during bank1], c0, c1, h0, h1 (bf16), cast hT0
    Pool: t2_0=f0*c_prev0, t2_1
  - 2 bf16 PE transposes of h halves -> hT stationaries for t+1.
  - hs[t] written bf16 (host upcasts).

Weight row permutation: PyTorch gate order (i,f,g,o) x 1024 cells ->
(j, g', c) with g' in (i, g~, o, f), j=cell//256, c=cell%256.
"""

import os
import sys

sys.path.insert(0, "/opt/trn_rl_repo")

from contextlib import ExitStack

import numpy as np

import concourse.bass as bass
import concourse.tile as tile
from concourse import bacc, mybir
from concourse.tile_rust import add_dep_helper

F32 = mybir.dt.float32
F32R = mybir.dt.float32r
BF16 = mybir.dt.bfloat16
AF = mybir.ActivationFunctionType

B_LOC = 16  # batch per core
H = 1024
I = 1024
KC = I // 128  # 8 contraction chunks
NG = 4  # column-tile groups
CPG = H // NG  # cells per group = 256
GATE_PERM = [0, 2, 3, 1]  # new gate order (i, g~, o, f) from pytorch (i, f, g, o)
KC_ORDER = [0, 2, 4, 6, 1, 3, 5, 7]  # even kc first (need only hT half 0)
LEAD = 2  # proj row-tiles computed ahead of consumption


# ----------------------------------------------------------------- host prep
def perm_rows(w4h: np.ndarray) -> np.ndarray:
    """Permute [4H, K] gate-major rows (pytorch i,f,g,o) -> (j, g', c) order."""
    k = w4h.shape[1]
    w = w4h.reshape(4, NG, CPG, k)[GATE_PERM]  # [g', j, c, K]
    w = w.transpose(1, 0, 2, 3)  # [j, g', c, K]
    return np.ascontiguousarray(w.reshape(4 * H, k))


def _bf16(a):
    import ml_dtypes
    return a.astype(ml_dtypes.bfloat16)


def prep_w_ih(w_ih: np.ndarray) -> np.ndarray:
    """[4H, I] -> [8, 128, 4H]  ([kc, p, n]) for SBUF rhs streaming."""
    wp = perm_rows(w_ih)  # [4096n, 1024i]
    return _bf16(np.ascontiguousarray(wp.T.reshape(I // 128, 128, 4 * H)))


def prep_w_hh(w_hh: np.ndarray) -> np.ndarray:
    """[4H, H] -> [8, 128, 4, 1024] ([kc, p, j, g'*c])."""
    wp = perm_rows(w_hh)  # [4096n=(j,g',c), 1024k]
    wt = wp.T.reshape(H // 128, 128, NG, 4 * CPG)
    return _bf16(np.ascontiguousarray(wt))


def prep_bias(b: np.ndarray) -> np.ndarray:
    return np.ascontiguousarray(perm_rows(b.reshape(4 * H, 1)).reshape(4 * H))


def prep_x(x_shard: np.ndarray, reverse_time: bool) -> np.ndarray:
    """[16, S, 1024] -> xT [1024, S*16] (i, t*b) t-major."""
    if reverse_time:
        x_shard = x_shard[:, ::-1, :]
    s = x_shard.shape[1]
    xt = x_shard.transpose(2, 1, 0)  # [i, t, b]
    return _bf16(np.ascontiguousarray(xt.reshape(I, s * B_LOC)))


# ------------------------------------------------------------------- builder
def build_program(S: int) -> bacc.Bacc:
    nc = bacc.Bacc(
        "TRN2",
        target_bir_lowering=False,
        debug=False,
        enable_asserts=True,
    )

    TB = S * B_LOC  # rows of the proj GEMM
    assert TB % 128 == 0
    NMT = TB // 128  # proj row tiles (= S // 8)
    assert NMT * 8 == S

    XPDT = BF16 if os.environ.get("LSTM_XP", "bf16") == "bf16" else F32R

    xT = nc.dram_tensor("xT", [I, TB], BF16, kind="ExternalInput").ap()
    w_ih = nc.dram_tensor("w_ih", [KC, 128, 4 * H], BF16, kind="ExternalInput").ap()
    w_hh = nc.dram_tensor("w_hh", [KC, 128, NG, 4 * CPG], BF16, kind="ExternalInput").ap()
    bias = nc.dram_tensor("bias", [4 * H], F32, kind="ExternalInput").ap()
    ident = nc.dram_tensor("ident", [32, 32], F32, kind="ExternalInput").ap()
    hs = nc.dram_tensor("hs", [S, B_LOC, H], BF16, kind="ExternalOutput").ap()
    # xp staged in DRAM: [t, j, b, g', c] single plane
    xp_d = nc.dram_tensor("xp_stage", [S, NG, B_LOC, 4, CPG], XPDT, kind="Internal").ap()
    XPST = NG * B_LOC * 4 * CPG  # xp elements per t

    with tile.TileContext(nc) as tc, ExitStack() as ctx:
        # PE executes serially; pin the scheduler to our emission order for all
        # PE instructions so psum accumulation-group start/stop semantics can't
        # be violated by hoisting (scheduling-only deps, no semaphores).
        pe_prev = [None]

        def pe(bi):
            if pe_prev[0] is not None:
                add_dep_helper(bi.ins, pe_prev[0].ins, sync=False,
                               reason="PE emission order")
            pe_prev[0] = bi
            return bi

        # =================== constants that live for the whole kernel ======
        const_pool = ctx.enter_context(tc.tile_pool(name="consts", bufs=1))

        # full 128x128 identity (bf16) for PE transpose (block-diagonal eye32)
        ident_f32 = const_pool.tile([128, 128], F32)
        nc.vector.memset(ident_f32[:], 0.0)
        for j in range(NG):
            nc.sync.dma_start(
                out=ident_f32[32 * j : 32 * (j + 1), 32 * j : 32 * (j + 1)],
                in_=ident,
            )
        ident_full = const_pool.tile([128, 128], BF16)
        nc.vector.tensor_copy(ident_full[:], ident_f32[:])
        # combined selection matrix: esel[16j+b, 32j+b] = 1 (b < 16), f32r.
        # One full-width MM injects xp into all 4 psum groups and opens the
        # accumulation group for the whole bank in a single start=True.
        esel_f32 = const_pool.tile([64, 128], F32)
        nc.vector.memset(esel_f32[:], 0.0)
        for j in range(NG):
            nc.sync.dma_start(
                out=esel_f32[16 * j : 16 * (j + 1), 32 * j : 32 * (j + 1)],
                in_=ident[0:16, :],
            )
        esel_r = const_pool.tile([64, 128], XPDT)
        nc.vector.tensor_copy(esel_r[:], esel_f32[:])

        # ======================= persistent weights ========================
        wih_sb = const_pool.tile([128, KC, 4 * H], BF16)
        for kc in range(KC):
            nc.sync.dma_start(out=wih_sb[:, kc, :], in_=w_ih[kc])
        whh_sb = const_pool.tile([128, KC, NG, 4 * CPG], BF16)
        for kc in range(KC):
            nc.sync.dma_start(out=whh_sb[:, kc, :, :], in_=w_hh[kc])
        bias_sb = const_pool.tile([128, 4 * H], F32)
        nc.sync.dma_start(
            out=bias_sb[:],
            in_=bass.AP(tensor=bias.tensor, offset=0, ap=[[0, 128], [1, 4 * H]]),
        )

        # ============================ pools ================================
        xt_pool = ctx.enter_context(tc.tile_pool(name="xt", bufs=2))
        pj_psum = ctx.enter_context(tc.tile_pool(name="pj_ps", bufs=2, space="PSUM"))
        pj_stage = ctx.enter_context(tc.tile_pool(name="pj_st", bufs=3))
        xp_pool = ctx.enter_context(tc.tile_pool(name="xp", bufs=3))
        g_psum = ctx.enter_context(tc.tile_pool(name="gates_ps", bufs=4, space="PSUM"))
        t_psum = ctx.enter_context(tc.tile_pool(name="tr_ps", bufs=2, space="PSUM"))
        a_pool = ctx.enter_context(tc.tile_pool(name="acts", bufs=2))
        c_pool = ctx.enter_context(tc.tile_pool(name="cell", bufs=2))
        h_pool = ctx.enter_context(tc.tile_pool(name="hid", bufs=2))
        tmp_pool = ctx.enter_context(tc.tile_pool(name="tmp", bufs=2))
        hT_pool = ctx.enter_context(tc.tile_pool(name="hT", bufs=4))

        # ======================= proj chunk helpers ========================
        xt_cur = [None]  # current row-tile's xT chunks [128, KC*128]

        def load_xt(mt):
            xt_t = xt_pool.tile([128, KC * 128], BF16, tag="xt")
            for kc in range(KC):
                nc.sync.dma_start(
                    out=xt_t[:, kc * 128 : (kc + 1) * 128],
                    in_=xT[kc * 128 : (kc + 1) * 128, mt * 128 : (mt + 1) * 128],
                )
            xt_cur[0] = xt_t

        def emit_proj_mms(mt, n8, kcs, ps=None):
            """kcs-slice of the 8 kc MMs of one [128,512] chunk of row-tile mt."""
            if ps is None:
                ps = pj_psum.tile([128, 512], F32, tag="pjps")
            for kc in kcs:
                pe(nc.tensor.matmul(
                    ps[:],
                    xt_cur[0][:, kc * 128 : (kc + 1) * 128],
                    wih_sb[:, kc, n8 * 512 : (n8 + 1) * 512],
                    start=(kc == 0),
                    stop=(kc == KC - 1),
                    skip_group_check=True,
                ))
            return ps

        def emit_proj_store(mt, n8, ps):
            stg = pj_stage.tile([128, 512], XPDT, tag="pjstg")
            nc.vector.tensor_add(
                stg[:], ps[:], bias_sb[:, n8 * 512 : (n8 + 1) * 512]
            )
            t0 = mt * 8  # first t of this row-tile (8 t's x 16 b's)
            j, g0 = n8 // 2, (n8 % 2) * 2
            dst = bass.AP(
                tensor=xp_d.tensor,
                offset=t0 * XPST + j * (B_LOC * 4 * CPG) + g0 * CPG,
                ap=[
                    [XPST, 8],  # t
                    [4 * CPG, B_LOC],  # b
                    [CPG, 2],  # g'
                    [1, CPG],  # c
                ],
            )
            src = bass.AP(
                tensor=stg.tensor,
                offset=stg[:].offset,
                ap=[[512, 128], [CPG, 2], [1, CPG]],
            )
            nc.sync.dma_start(out=dst, in_=src)

        def inject_xp(t):
            """Load xp(t) and open the psum pair for step t with it."""
            xp_t = xp_pool.tile([64, 4 * CPG], XPDT, tag="xp")
            nc.sync.dma_start(
                out=xp_t[:],
                in_=bass.AP(
                    tensor=xp_d.tensor,
                    offset=t * XPST,
                    ap=[[4 * CPG, NG * B_LOC], [1, 4 * CPG]],
                ),
            )
            Gb = [g_psum.tile([128, 512], F32, tag="G", name=f"G{nh}")
                  for nh in range(2)]
            for nh in range(2):
                pe(nc.tensor.matmul(
                    Gb[nh][:],
                    esel_r[:],
                    xp_t[:, nh * 512 : (nh + 1) * 512],
                    start=True,
                    stop=(t == 0),  # t=0 has no h-MMs
                    skip_group_check=True,
                ))
            return Gb

        # ========================= prologue ================================
        # proj row-tiles 0..LEAD-1 so xp(t) exists for the first 8*LEAD steps
        for mt in range(LEAD):
            load_xt(mt)
            for n8 in range(8):
                ps = emit_proj_mms(mt, n8, range(KC))
                emit_proj_store(mt, n8, ps)

        next_Gb = inject_xp(0)

        # ========================= the scan ================================
        c_prev = None
        hT_prev = None  # [hT_half0, hT_half1] stationary tiles [128,128] bf16
        for t in range(S):
            Gb = next_Gb

            # ---- h-MMs: bank-major, even kc chunks first
            if hT_prev is not None:
                for nh in range(2):
                    for kci, kc in enumerate(KC_ORDER):
                        jc, half = kc // 2, kc % 2
                        for j in range(NG):
                            pe(nc.tensor.matmul(
                                Gb[nh][32 * j : 32 * (j + 1), :],
                                hT_prev[half][:, 32 * jc : 32 * (jc + 1)],
                                whh_sb[:, kc, j, nh * 512 : (nh + 1) * 512],
                                start=False,
                                stop=(kci == KC - 1),
                                tile_position=(0, 32 * j),
                                skip_group_check=True,
                            ))

            # ---- bank0 activations (overlap bank1 h-MMs): i sigmoid, g~ tanh
            A = a_pool.tile([128, 4 * CPG], F32, tag="A")
            nc.scalar.activation(A[:, 0:CPG], Gb[0][:, 0:CPG], AF.Sigmoid)
            nc.scalar.activation(A[:, CPG : 2 * CPG], Gb[0][:, CPG : 2 * CPG], AF.Tanh)
            t1 = tmp_pool.tile([128, CPG], F32, tag="T1")
            nc.gpsimd.tensor_mul(t1[:], A[:, 0:CPG], A[:, CPG : 2 * CPG])

            # ---- open next psum pair with xp(t+1); proj part A fills the
            # PE gap until h half 0 is ready for its transpose
            if t + 1 < S:
                next_Gb = inject_xp(t + 1)
            ck = t // 8 + LEAD
            pj_ps = None
            if ck < NMT:
                n8 = t % 8
                if n8 == 0:
                    load_xt(ck)
                pj_ps = emit_proj_mms(ck, n8, range(0, 6))

            # ---- bank1 activations: f halves first (feed c chain)
            af = A[:, 3 * CPG : 4 * CPG]
            nc.scalar.activation(A[:, 3 * CPG : 3 * CPG + 128],
                                 Gb[1][:, CPG : CPG + 128], AF.Sigmoid)
            nc.scalar.activation(A[:, 3 * CPG + 128 : 4 * CPG],
                                 Gb[1][:, CPG + 128 : 2 * CPG], AF.Sigmoid)

            # ---- cell/hidden update, split into cell halves; DVE chain runs
            # t2/c/h back-to-back; ACT runs o-half then tanh(c-half)
            c_new = c_pool.tile([128, CPG], F32, tag="C")
            tc_t = tmp_pool.tile([128, CPG], F32, tag="TC")
            h_new = h_pool.tile([128, CPG], BF16, tag="Hb")
            t2 = tmp_pool.tile([128, CPG], F32, tag="T2")
            pts = []
            for hf in range(2):
                sl = slice(128 * hf, 128 * (hf + 1))
                ao_h = A[:, 2 * CPG + 128 * hf : 2 * CPG + 128 * (hf + 1)]
                nc.scalar.activation(ao_h, Gb[1][:, 128 * hf : 128 * (hf + 1)],
                                     AF.Sigmoid)
                if c_prev is None:
                    nc.vector.tensor_copy(c_new[:, sl], t1[:, sl])
                else:
                    nc.vector.tensor_mul(t2[:, sl], af[:, sl], c_prev[:, sl])
                    nc.vector.tensor_add(c_new[:, sl], t1[:, sl], t2[:, sl])
                nc.scalar.activation(tc_t[:, sl], c_new[:, sl], AF.Tanh)
                nc.vector.tensor_mul(h_new[:, sl], ao_h, tc_t[:, sl])
                # transpose this half for next step's stationary operand
                if t < S - 1:
                    pt = t_psum.tile([128, 128], BF16, tag="PT")
                    pe(nc.tensor.transpose(pt[:], h_new[:, sl], ident_full[:]))
                    pts.append(pt)
                    if hf == 0 and pj_ps is not None:
                        # one proj MM bridges the transp0 -> transp1 gap
                        emit_proj_mms(ck, t % 8, range(6, 7), pj_ps)
            c_prev = c_new

            # ---- last proj MM rides after the transposes
            if pj_ps is not None:
                emit_proj_mms(ck, t % 8, range(7, 8), pj_ps)

            # ---- hT psum->SBUF copies: cast0 on DVE (emitted after the h
            # chain so it can't block it), copy1 on ACT
            if t < S - 1:
                ht0 = hT_pool.tile([128, 128], BF16, tag="hT", name="hT0")
                nc.vector.tensor_copy(ht0[:], pts[0][:])
                ht1 = hT_pool.tile([128, 128], BF16, tag="hT", name="hT1")
                nc.scalar.copy(ht1[:], pts[1][:])
                hT_prev = [ht0, ht1]

            # ---- proj chunk epilogue (bias add + store), DVE last
            if pj_ps is not None:
                emit_proj_store(ck, t % 8, pj_ps)

            # ---- write hs[t]  (group j partitions -> hs[t, :, 256j:256j+256])
            for j in range(NG):
                nc.sync.dma_start(
                    out=bass.AP(
                        tensor=hs.tensor,
                        offset=t * (B_LOC * H) + j * CPG,
                        ap=[[H, B_LOC], [1, CPG]],
                    ),
                    in_=h_new[32 * j : 32 * j + 16, :],
                )

    nc.compile()
    return nc


# ------------------------------------------------------------------ wrapper
def make_in_maps(x, W_ih_fw, W_hh_fw, b_fw, W_ih_bw, W_hh_bw, b_bw, bw_h_mask):
    """Full inputs -> list of 8 per-core input dicts."""
    x = np.asarray(x, np.float32)
    eye = np.eye(32, dtype=np.float32)
    whh_bw_m = np.asarray(W_hh_bw, np.float32) * np.asarray(
        bw_h_mask, np.float32
    ).reshape(1, H)
    fw = dict(
        w_ih=prep_w_ih(np.asarray(W_ih_fw, np.float32)),
        w_hh=prep_w_hh(np.asarray(W_hh_fw, np.float32)),
        bias=prep_bias(np.asarray(b_fw, np.float32)),
    )
    bw = dict(
        w_ih=prep_w_ih(np.asarray(W_ih_bw, np.float32)),
        w_hh=prep_w_hh(whh_bw_m),
        bias=prep_bias(np.asarray(b_bw, np.float32)),
    )
    in_maps = []
    for core in range(8):
        rev = core >= 4
        wd = bw if rev else fw
        bc = (core % 4) * B_LOC
        in_maps.append(
            dict(
                xT=prep_x(x[bc : bc + B_LOC], rev),
                ident=eye,
                **wd,
            )
        )
    return in_maps


def assemble_output(results, S):
    """8 per-core 'hs' [S,16,H] (bf16) -> [B, S, 2H] f32."""
    out = np.empty((64, S, 2 * H), np.float32)
    for core in range(8):
        hs = np.asarray(results[core]["hs"]).astype(np.float32)
        bc = (core % 4) * B_LOC
        if core < 4:
            out[bc : bc + B_LOC, :, :H] = hs.transpose(1, 0, 2)
        else:
            out[bc : bc + B_LOC, :, H:] = hs[::-1].transpose(1, 0, 2)
    return out


# ======================================================================
# harness entry point
# ======================================================================
_PROG_CACHE = {}


def _get_program(S):
    if S not in _PROG_CACHE:
        _PROG_CACHE[S] = build_program(S)
    return _PROG_CACHE[S]


def kernel(x, W_ih_fw, W_hh_fw, b_fw, W_ih_bw, W_hh_bw, b_bw, bw_h_mask):
    """Full (unsharded) inputs -> full [B, S, 2H] output on 8 TRN2
    NeuronCores (cores 0-3 forward, 4-7 backward, batch-sharded by 16)."""
    from concourse.bass_utils import run_bass_kernel_spmd

    x = np.asarray(x, np.float32)
    S = x.shape[1]
    nc = _get_program(S)
    in_maps = make_in_maps(
        x, W_ih_fw, W_hh_fw, b_fw, W_ih_bw, W_hh_bw, b_bw, bw_h_mask
    )
    res = run_bass_kernel_spmd(nc, in_maps, core_ids=list(range(8)))
    return assemble_output(res.results, S)


# revision 10
# speedup vs baseline: 1.0886x; 1.0286x over previous
"""Bidirectional LSTM layer on 8 TRN2 NeuronCores (Bass/Tile).

Problem: B=64, S=512, I=H=1024, fp32.
  hs_fw = LSTM_fw(x), hs_bw = reverse(LSTM_bw(reverse(x))), out = concat -> [B,S,2H]

Sharding: pure SPMD, one program. Cores 0-3: forward dir, batch chunks of 16.
Cores 4-7: backward dir (host feeds time-reversed x, host un-reverses output),
batch chunks of 16. bw_h_mask is folded into W_hh_bw on the host.

Single fused loop (v2): the input projection GEMM is interleaved into the
recurrent scan one [128,512] output chunk per step (8 kc matmuls), so the PE
fills the elementwise-tail gap and stays at full pump (HAM K=8/8).

Per step t:
  - 64 h-MMs: gates += hT(t-1) @ W_hh', 4-way PE column tiling (groups j own
    psum partitions 32j..32j+32), bank-major (bank0 = [i|g~], bank1 = [o|f]),
    even kc chunks first (they only need hT half 0).
  - 2 f32r inject MMs put xp(t+1) into the next psum pair (start=True).
  - 8 proj MMs: one n8-chunk of row-tile t//8 + LEAD; bias-add on DVE; staged
    to DRAM f32r.
  - elementwise tail split into cell halves so transpose(h half0) can feed
    step t+1's first (even-kc) h-MMs while half1 finishes:
    ACT: sig(i), tanh(g~) [during bank1 MMs], sig(f0), sig(f1), sig(o),
         tanh(c0), tanh(c1), copy hT1
    DVE: t1=i*g~ [during bank1], c0, c1, h0, h1 (bf16), cast hT0
    Pool: t2_0=f0*c_prev0, t2_1
  - 2 bf16 PE transposes of h halves -> hT stationaries for t+1.
  - hs[t] written bf16 (host upcasts).

Weight row permutation: PyTorch gate order (i,f,g,o) x 1024 cells ->
(j, g', c) with g' in (i, g~, o, f), j=cell//256, c=cell%256.
"""

import os
import sys

sys.path.insert(0, "/opt/trn_rl_repo")

from contextlib import ExitStack

import numpy as np

import concourse.bass as bass
import concourse.tile as tile
from concourse import bacc, mybir
from concourse.tile_rust import add_dep_helper

F32 = mybir.dt.float32
F32R = mybir.dt.float32r
BF16 = mybir.dt.bfloat16
AF = mybir.ActivationFunctionType

B_LOC = 16  # batch per core
H = 1024
I = 1024
KC = I // 128  # 8 contraction chunks
NG = 4  # column-tile groups
CPG = H // NG  # cells per group = 256
GATE_PERM = [0, 2, 3, 1]  # new gate order (i, g~, o, f) from pytorch (i, f, g, o)
KC_ORDER = [0, 2, 4, 6, 1, 3, 5, 7]  # even kc first (need only hT half 0)
LEAD = 2  # proj row-tiles computed ahead of consumption


# ----------------------------------------------------------------- host prep
def perm_rows(w4h: np.ndarray) -> np.ndarray:
    """Permute [4H, K] gate-major rows (pytorch i,f,g,o) -> (j, g', c) order."""
    k = w4h.shape[1]
    w = w4h.reshape(4, NG, CPG, k)[GATE_PERM]  # [g', j, c, K]
    w = w.transpose(1, 0, 2, 3)  # [j, g', c, K]
    return np.ascontiguousarray(w.reshape(4 * H, k))


def _bf16(a):
    import ml_dtypes
    return a.astype(ml_dtypes.bfloat16)


def prep_w_ih(w_ih: np.ndarray) -> np.ndarray:
    """[4H, I] -> [8, 128, 4H]  ([kc, p, n]) for SBUF rhs streaming."""
    wp = perm_rows(w_ih)  # [4096n, 1024i]
    return _bf16(np.ascontiguousarray(wp.T.reshape(I // 128, 128, 4 * H)))


def prep_w_hh(w_hh: np.ndarray) -> np.ndarray:
    """[4H, H] -> [8, 128, 4, 1024] ([kc, p, j, g'*c])."""
    wp = perm_rows(w_hh)  # [4096n=(j,g',c), 1024k]
    wt = wp.T.reshape(H // 128, 128, NG, 4 * CPG)
    return _bf16(np.ascontiguousarray(wt))


def prep_bias(b: np.ndarray) -> np.ndarray:
    return np.ascontiguousarray(perm_rows(b.reshape(4 * H, 1)).reshape(4 * H))


def prep_x(x_shard: np.ndarray, reverse_time: bool) -> np.ndarray:
    """[16, S, 1024] -> xT [1024, S*16] (i, t*b) t-major."""
    if reverse_time:
        x_shard = x_shard[:, ::-1, :]
    s = x_shard.shape[1]
    xt = x_shard.transpose(2, 1, 0)  # [i, t, b]
    return _bf16(np.ascontiguousarray(xt.reshape(I, s * B_LOC)))


# ------------------------------------------------------------------- builder
def build_program(S: int) -> bacc.Bacc:
    nc = bacc.Bacc(
        "TRN2",
        target_bir_lowering=False,
        debug=False,
        enable_asserts=True,
    )

    TB = S * B_LOC  # rows of the proj GEMM
    assert TB % 128 == 0
    NMT = TB // 128  # proj row tiles (= S // 8)
    assert NMT * 8 == S

    XPDT = BF16 if os.environ.get("LSTM_XP", "bf16") == "bf16" else F32R

    xT = nc.dram_tensor("xT", [I, TB], BF16, kind="ExternalInput").ap()
    w_ih = nc.dram_tensor("w_ih", [KC, 128, 4 * H], BF16, kind="ExternalInput").ap()
    w_hh = nc.dram_tensor("w_hh", [KC, 128, NG, 4 * CPG], BF16, kind="ExternalInput").ap()
    bias = nc.dram_tensor("bias", [4 * H], F32, kind="ExternalInput").ap()
    ident = nc.dram_tensor("ident", [32, 32], F32, kind="ExternalInput").ap()
    hs = nc.dram_tensor("hs", [S, B_LOC, H], BF16, kind="ExternalOutput").ap()
    # xp staged in DRAM: [t, j, b, g', c] single plane
    xp_d = nc.dram_tensor("xp_stage", [S, NG, B_LOC, 4, CPG], XPDT, kind="Internal").ap()
    XPST = NG * B_LOC * 4 * CPG  # xp elements per t

    with tile.TileContext(nc) as tc, ExitStack() as ctx:
        # PE executes serially; pin the scheduler to our emission order for all
        # PE instructions so psum accumulation-group start/stop semantics can't
        # be violated by hoisting (scheduling-only deps, no semaphores).
        pe_prev = [None]

        def pe(bi):
            if pe_prev[0] is not None:
                add_dep_helper(bi.ins, pe_prev[0].ins, sync=False,
                               reason="PE emission order")
            pe_prev[0] = bi
            return bi

        # =================== constants that live for the whole kernel ======
        const_pool = ctx.enter_context(tc.tile_pool(name="consts", bufs=1))

        # full 128x128 identity (bf16) for PE transpose (block-diagonal eye32)
        ident_f32 = const_pool.tile([128, 128], F32)
        nc.vector.memset(ident_f32[:], 0.0)
        for j in range(NG):
            nc.sync.dma_start(
                out=ident_f32[32 * j : 32 * (j + 1), 32 * j : 32 * (j + 1)],
                in_=ident,
            )
        ident_full = const_pool.tile([128, 128], BF16)
        nc.vector.tensor_copy(ident_full[:], ident_f32[:])
        # combined selection matrix: esel[16j+b, 32j+b] = 1 (b < 16), f32r.
        # One full-width MM injects xp into all 4 psum groups and opens the
        # accumulation group for the whole bank in a single start=True.
        esel_f32 = const_pool.tile([64, 128], F32)
        nc.vector.memset(esel_f32[:], 0.0)
        for j in range(NG):
            nc.sync.dma_start(
                out=esel_f32[16 * j : 16 * (j + 1), 32 * j : 32 * (j + 1)],
                in_=ident[0:16, :],
            )
        esel_r = const_pool.tile([64, 128], XPDT)
        nc.vector.tensor_copy(esel_r[:], esel_f32[:])

        # ======================= persistent weights ========================
        wih_sb = const_pool.tile([128, KC, 4 * H], BF16)
        for kc in range(KC):
            nc.sync.dma_start(out=wih_sb[:, kc, :], in_=w_ih[kc])
        whh_sb = const_pool.tile([128, KC, NG, 4 * CPG], BF16)
        for kc in range(KC):
            nc.sync.dma_start(out=whh_sb[:, kc, :, :], in_=w_hh[kc])
        bias_sb = const_pool.tile([128, 4 * H], F32)
        nc.sync.dma_start(
            out=bias_sb[:],
            in_=bass.AP(tensor=bias.tensor, offset=0, ap=[[0, 128], [1, 4 * H]]),
        )

        # ============================ pools ================================
        xt_pool = ctx.enter_context(tc.tile_pool(name="xt", bufs=2))
        pj_psum = ctx.enter_context(tc.tile_pool(name="pj_ps", bufs=2, space="PSUM"))
        pj_stage = ctx.enter_context(tc.tile_pool(name="pj_st", bufs=3))
        xp_pool = ctx.enter_context(tc.tile_pool(name="xp", bufs=3))
        g_psum = ctx.enter_context(tc.tile_pool(name="gates_ps", bufs=4, space="PSUM"))
        t_psum = ctx.enter_context(tc.tile_pool(name="tr_ps", bufs=2, space="PSUM"))
        a_pool = ctx.enter_context(tc.tile_pool(name="acts", bufs=2))
        c_pool = ctx.enter_context(tc.tile_pool(name="cell", bufs=2))
        h_pool = ctx.enter_context(tc.tile_pool(name="hid", bufs=2))
        tmp_pool = ctx.enter_context(tc.tile_pool(name="tmp", bufs=2))
        hT_pool = ctx.enter_context(tc.tile_pool(name="hT", bufs=4))

        # ======================= proj chunk helpers ========================
        xt_cur = [None]  # current row-tile's xT chunks [128, KC*128]

        def load_xt(mt):
            xt_t = xt_pool.tile([128, KC * 128], BF16, tag="xt")
            for kc in range(KC):
                nc.sync.dma_start(
                    out=xt_t[:, kc * 128 : (kc + 1) * 128],
                    in_=xT[kc * 128 : (kc + 1) * 128, mt * 128 : (mt + 1) * 128],
                )
            xt_cur[0] = xt_t

        def emit_proj_mms(mt, n8, kcs, ps=None):
            """kcs-slice of the 8 kc MMs of one [128,512] chunk of row-tile mt."""
            if ps is None:
                ps = pj_psum.tile([128, 512], F32, tag="pjps")
            for kc in kcs:
                pe(nc.tensor.matmul(
                    ps[:],
                    xt_cur[0][:, kc * 128 : (kc + 1) * 128],
                    wih_sb[:, kc, n8 * 512 : (n8 + 1) * 512],
                    start=(kc == 0),
                    stop=(kc == KC - 1),
                    skip_group_check=True,
                ))
            return ps

        def emit_proj_store(mt, n8, ps):
            stg = pj_stage.tile([128, 512], XPDT, tag="pjstg")
            nc.vector.tensor_add(
                stg[:], ps[:], bias_sb[:, n8 * 512 : (n8 + 1) * 512]
            )
            t0 = mt * 8  # first t of this row-tile (8 t's x 16 b's)
            j, g0 = n8 // 2, (n8 % 2) * 2
            dst = bass.AP(
                tensor=xp_d.tensor,
                offset=t0 * XPST + j * (B_LOC * 4 * CPG) + g0 * CPG,
                ap=[
                    [XPST, 8],  # t
                    [4 * CPG, B_LOC],  # b
                    [CPG, 2],  # g'
                    [1, CPG],  # c
                ],
            )
            src = bass.AP(
                tensor=stg.tensor,
                offset=stg[:].offset,
                ap=[[512, 128], [CPG, 2], [1, CPG]],
            )
            nc.sync.dma_start(out=dst, in_=src)

        def inject_xp(t):
            """Load xp(t) and open the psum pair for step t with it."""
            xp_t = xp_pool.tile([64, 4 * CPG], XPDT, tag="xp")
            nc.sync.dma_start(
                out=xp_t[:],
                in_=bass.AP(
                    tensor=xp_d.tensor,
                    offset=t * XPST,
                    ap=[[4 * CPG, NG * B_LOC], [1, 4 * CPG]],
                ),
            )
            Gb = [g_psum.tile([128, 512], F32, tag="G", name=f"G{nh}")
                  for nh in range(2)]
            for nh in range(2):
                pe(nc.tensor.matmul(
                    Gb[nh][:],
                    esel_r[:],
                    xp_t[:, nh * 512 : (nh + 1) * 512],
                    start=True,
                    stop=(t == 0),  # t=0 has no h-MMs
                    skip_group_check=True,
                ))
            return Gb

        # ========================= prologue ================================
        # proj row-tiles 0..LEAD-1 so xp(t) exists for the first 8*LEAD steps
        for mt in range(LEAD):
            load_xt(mt)
            for n8 in range(8):
                ps = emit_proj_mms(mt, n8, range(KC))
                emit_proj_store(mt, n8, ps)

        next_Gb = inject_xp(0)

        # ========================= the scan ================================
        c_prev = None
        hT_prev = None  # [hT_half0, hT_half1] stationary tiles [128,128] bf16
        for t in range(S):
            Gb = next_Gb

            # ---- h-MMs: bank-major, even kc chunks first
            if hT_prev is not None:
                for nh in range(2):
                    for kci, kc in enumerate(KC_ORDER):
                        jc, half = kc // 2, kc % 2
                        for j in range(NG):
                            pe(nc.tensor.matmul(
                                Gb[nh][32 * j : 32 * (j + 1), :],
                                hT_prev[half][:, 32 * jc : 32 * (jc + 1)],
                                whh_sb[:, kc, j, nh * 512 : (nh + 1) * 512],
                                start=False,
                                stop=(kci == KC - 1),
                                tile_position=(0, 32 * j),
                                skip_group_check=True,
                            ))

            # ---- bank0 activations (overlap bank1 h-MMs): i sigmoid, g~ tanh
            A = a_pool.tile([128, 4 * CPG], F32, tag="A")
            nc.scalar.activation(A[:, 0:CPG], Gb[0][:, 0:CPG], AF.Sigmoid)
            nc.scalar.activation(A[:, CPG : 2 * CPG], Gb[0][:, CPG : 2 * CPG], AF.Tanh)
            t1 = tmp_pool.tile([128, CPG], F32, tag="T1")
            nc.gpsimd.tensor_mul(t1[:], A[:, 0:CPG], A[:, CPG : 2 * CPG])

            # ---- open next psum pair with xp(t+1); proj part A fills the
            # PE gap until h half 0 is ready for its transpose
            if t + 1 < S:
                next_Gb = inject_xp(t + 1)
            ck = t // 8 + LEAD
            pj_ps = None
            if ck < NMT:
                n8 = t % 8
                if n8 == 0:
                    load_xt(ck)
                pj_ps = emit_proj_mms(ck, n8, range(0, 6))

            # ---- bank1 activations: f halves first (feed c chain)
            af = A[:, 3 * CPG : 4 * CPG]
            nc.scalar.activation(A[:, 3 * CPG : 3 * CPG + 128],
                                 Gb[1][:, CPG : CPG + 128], AF.Sigmoid)
            nc.scalar.activation(A[:, 3 * CPG + 128 : 4 * CPG],
                                 Gb[1][:, CPG + 128 : 2 * CPG], AF.Sigmoid)

            # ---- cell/hidden update, split into cell halves; DVE chain runs
            # t2/c/h back-to-back; ACT runs o-half then tanh(c-half)
            c_new = c_pool.tile([128, CPG], F32, tag="C")
            tc_t = tmp_pool.tile([128, CPG], F32, tag="TC")
            h_new = h_pool.tile([128, CPG], BF16, tag="Hb")
            t2 = tmp_pool.tile([128, CPG], F32, tag="T2")
            pts = []
            for hf in range(2):
                sl = slice(128 * hf, 128 * (hf + 1))
                ao_h = A[:, 2 * CPG + 128 * hf : 2 * CPG + 128 * (hf + 1)]
                nc.scalar.activation(ao_h, Gb[1][:, 128 * hf : 128 * (hf + 1)],
                                     AF.Sigmoid)
                if c_prev is None:
                    nc.vector.tensor_copy(c_new[:, sl], t1[:, sl])
                else:
                    nc.vector.tensor_mul(t2[:, sl], af[:, sl], c_prev[:, sl])
                    nc.vector.tensor_add(c_new[:, sl], t1[:, sl], t2[:, sl])
                nc.scalar.activation(tc_t[:, sl], c_new[:, sl], AF.Tanh)
                nc.vector.tensor_mul(h_new[:, sl], ao_h, tc_t[:, sl])
                # one proj MM rides while this h half is computed, then the
                # transpose fires exactly when the half lands
                if pj_ps is not None:
                    emit_proj_mms(ck, t % 8, range(6 + hf, 7 + hf), pj_ps)
                if t < S - 1:
                    pt = t_psum.tile([128, 128], BF16, tag="PT")
                    pe(nc.tensor.transpose(pt[:], h_new[:, sl], ident_full[:]))
                    pts.append(pt)
            c_prev = c_new

            # ---- hT psum->SBUF copies: cast0 on DVE (emitted after the h
            # chain so it can't block it), copy1 on ACT
            if t < S - 1:
                ht0 = hT_pool.tile([128, 128], BF16, tag="hT", name="hT0")
                nc.vector.tensor_copy(ht0[:], pts[0][:])
                ht1 = hT_pool.tile([128, 128], BF16, tag="hT", name="hT1")
                nc.scalar.copy(ht1[:], pts[1][:])
                hT_prev = [ht0, ht1]

            # ---- proj chunk epilogue (bias add + store), DVE last
            if pj_ps is not None:
                emit_proj_store(ck, t % 8, pj_ps)

            # ---- write hs[t]  (group j partitions -> hs[t, :, 256j:256j+256])
            for j in range(NG):
                nc.sync.dma_start(
                    out=bass.AP(
                        tensor=hs.tensor,
                        offset=t * (B_LOC * H) + j * CPG,
                        ap=[[H, B_LOC], [1, CPG]],
                    ),
                    in_=h_new[32 * j : 32 * j + 16, :],
                )

    nc.compile()
    return nc


# ------------------------------------------------------------------ wrapper
def make_in_maps(x, W_ih_fw, W_hh_fw, b_fw, W_ih_bw, W_hh_bw, b_bw, bw_h_mask):
    """Full inputs -> list of 8 per-core input dicts."""
    x = np.asarray(x, np.float32)
    eye = np.eye(32, dtype=np.float32)
    whh_bw_m = np.asarray(W_hh_bw, np.float32) * np.asarray(
        bw_h_mask, np.float32
    ).reshape(1, H)
    fw = dict(
        w_ih=prep_w_ih(np.asarray(W_ih_fw, np.float32)),
        w_hh=prep_w_hh(np.asarray(W_hh_fw, np.float32)),
        bias=prep_bias(np.asarray(b_fw, np.float32)),
    )
    bw = dict(
        w_ih=prep_w_ih(np.asarray(W_ih_bw, np.float32)),
        w_hh=prep_w_hh(whh_bw_m),
        bias=prep_bias(np.asarray(b_bw, np.float32)),
    )
    in_maps = []
    for core in range(8):
        rev = core >= 4
        wd = bw if rev else fw
        bc = (core % 4) * B_LOC
        in_maps.append(
            dict(
                xT=prep_x(x[bc : bc + B_LOC], rev),
                ident=eye,
                **wd,
            )
        )
    return in_maps


def assemble_output(results, S):
    """8 per-core 'hs' [S,16,H] (bf16) -> [B, S, 2H] f32."""
    out = np.empty((64, S, 2 * H), np.float32)
    for core in range(8):
        hs = np.asarray(results[core]["hs"]).astype(np.float32)
        bc = (core % 4) * B_LOC
        if core < 4:
            out[bc : bc + B_LOC, :, :H] = hs.transpose(1, 0, 2)
        else:
            out[bc : bc + B_LOC, :, H:] = hs[::-1].transpose(1, 0, 2)
    return out


# ======================================================================
# harness entry point
# ======================================================================
_PROG_CACHE = {}


def _get_program(S):
    if S not in _PROG_CACHE:
        _PROG_CACHE[S] = build_program(S)
    return _PROG_CACHE[S]


def kernel(x, W_ih_fw, W_hh_fw, b_fw, W_ih_bw, W_hh_bw, b_bw, bw_h_mask):
    """Full (unsharded) inputs -> full [B, S, 2H] output on 8 TRN2
    NeuronCores (cores 0-3 forward, 4-7 backward, batch-sharded by 16)."""
    from concourse.bass_utils import run_bass_kernel_spmd

    x = np.asarray(x, np.float32)
    S = x.shape[1]
    nc = _get_program(S)
    in_maps = make_in_maps(
        x, W_ih_fw, W_hh_fw, b_fw, W_ih_bw, W_hh_bw, b_bw, bw_h_mask
    )
    res = run_bass_kernel_spmd(nc, in_maps, core_ids=list(range(8)))
    return assemble_output(res.results, S)
